# revision 3
# baseline (speedup 1.0000x reference)
"""Trainium2 Bass kernel for nn_Decoder (attention-LSTM decoder recurrence).

v2: latency-oriented redesign. Per core: 4 batches, T-1=127 serial steps.

Critical chain per step:
  h-mms (PE: W_dh.2h bias accumulated into UHe psum, broadcast rhs)
  -> e0/e1 tanh (ACT [128,128], no bias operand)
  -> lg mms (PE, ES stationary) -> exp (ACT) -> den mm (PE)
  -> r = num/den (DVE divide) -> mm_B gates y~-part (PE) -> tg tanh (ACT)
  -> sU/sV/cs (Pool) -> th tanh (ACT) -> syh (Pool) -> next h-mms

Off-chain: UH recompute + c-part mms (PE), gates h/dec/bias part (mm_A, PE),
num mms (PE), decw copy (Pool).

e-layout: [(j,m) partitions, (g,t) free]; the attention bias W_d.q is
accumulated by PE into the e-input psum (rhs = state column broadcast over
t), eliminating the PSUM->SBUF bias copies of the baseline.

State: sy [66, BL] = [2h(0:64); w~0*dec_t(64); 1(65)], cs [64, BL] = 2c.
sigmoid(z) = 0.5 tanh(0.5 z) + 0.5 folded into packed weights.
"""

import numpy as np

B, T, M, P = 32, 128, 64, 64
NCORES = 8
BL = B // NCORES          # batches per core = 4
NG = 2                    # attention groups (pairs of batches)

_STATE = {}


def _build_nc():
    import contextlib

    import concourse.bacc as bacc
    import concourse.tile as tile
    from concourse import mybir

    f32 = mybir.dt.float32
    f32r = mybir.dt.float32r
    f16 = mybir.dt.float16
    AF = mybir.ActivationFunctionType
    OP = mybir.AluOpType

    nc = bacc.Bacc()

    # ---- per-core sharded data ----
    h_l = nc.declare_dram_parameter("h_l", [T, BL * M], f32, isOutput=False)
    hts = nc.declare_dram_parameter("hts", [M, 4 * T], f16, isOutput=False)
    dec_l = nc.declare_dram_parameter("dec_l", [1, BL * T], f32, isOutput=False)
    st0 = nc.declare_dram_parameter("st0", [2 * P, BL], f16, isOutput=False)
    # ---- replicated packed weights ----
    wdsuite = nc.declare_dram_parameter("wdsuite", [M, 12 * M], f16, isOutput=False)
    v2 = nc.declare_dram_parameter("v2", [2 * M, NG], f16, isOutput=False)
    wt1h = nc.declare_dram_parameter("wt1h", [M, 1], f16, isOutput=False)
    wtbbc = nc.declare_dram_parameter("wtbbc", [T, 1], f32, isOutput=False)
    whha = nc.declare_dram_parameter("whha", [P + 2, 4 * P], f16, isOutput=False)
    wihb = nc.declare_dram_parameter("wihb", [1, 4 * P], f16, isOutput=False)
    # ---- outputs ----
    oh = nc.declare_dram_parameter("oh", [P, BL], f32, isOutput=True)
    octx = nc.declare_dram_parameter("octx", [M, BL], f32, isOutput=True)
    oden = nc.declare_dram_parameter("oden", [1, BL], f32, isOutput=True)

    with tile.TileContext(nc) as tc:
        with contextlib.ExitStack() as ctx:
            consts = ctx.enter_context(tc.tile_pool(name="consts", bufs=1))
            state = ctx.enter_context(tc.tile_pool(name="state", bufs=1))
            loop_sb = ctx.enter_context(tc.tile_pool(name="loop_sb", bufs=1))
            ps_uhe0 = ctx.enter_context(
                tc.tile_pool(name="ps_uhe0", bufs=1, space="PSUM"))
            ps_uhe1 = ctx.enter_context(
                tc.tile_pool(name="ps_uhe1", bufs=1, space="PSUM"))
            ps_lg = ctx.enter_context(
                tc.tile_pool(name="ps_lg", bufs=1, space="PSUM"))
            ps_dn = ctx.enter_context(
                tc.tile_pool(name="ps_dn", bufs=1, space="PSUM"))
            ps_g = ctx.enter_context(
                tc.tile_pool(name="ps_g", bufs=1, space="PSUM"))
            ps_aux = ctx.enter_context(
                tc.tile_pool(name="ps_aux", bufs=1, space="PSUM"))

            # ---------------- constants ----------------
            # DMAs spread across SP/ACT/DVE queues; critical-path first.
            hts_sb = consts.tile([M, 4 * T], f16)
            nc.sync.dma_start(out=hts_sb, in_=hts[:])
            hte_sb = hts_sb[:, 0:2 * T]
            hto_sb = hts_sb[:, 2 * T:4 * T]
            wdsuite_sb = consts.tile([M, 12 * M], f16)
            nc.scalar.dma_start(out=wdsuite_sb, in_=wdsuite[:])
            udTp_sb = wdsuite_sb[:, 0:4 * M]
            wdh2_sb = wdsuite_sb[:, 4 * M:8 * M]
            wdc2_sb = wdsuite_sb[:, 8 * M:12 * M]
            st0_sb_dma = None
            v2_sb = consts.tile([2 * M, NG], f16)
            nc.gpsimd.dma_start(out=v2_sb, in_=v2[:])
            wt1h_sb = consts.tile([M, 1], f16)
            nc.gpsimd.dma_start(out=wt1h_sb, in_=wt1h[:])
            wtb_sb = consts.tile([T, 1], f32)
            nc.gpsimd.dma_start(out=wtb_sb, in_=wtbbc[:])
            whha_sb = consts.tile([P + 2, 4 * P], f16)
            nc.scalar.dma_start(out=whha_sb, in_=whha[:])
            wihb_sb = consts.tile([1, 4 * P], f16)
            nc.scalar.dma_start(out=wihb_sb, in_=wihb[:])
            ones_f = consts.tile([T, 1], f32)
            nc.vector.memset(ones_f, 1.0)
            ones_col = consts.tile([T, 1], f16)
            nc.vector.tensor_copy(out=ones_col, in_=ones_f)
            # dec (pre-multiplied by w~0 on host) lives at partition row 64
            decw_sb = consts.tile([66, BL * T], f32)
            nc.gpsimd.dma_start(out=decw_sb[64:65, :], in_=dec_l[:])
            decw16 = consts.tile([66, BL * T], f16)
            nc.vector.memset(decw16[64:66, :], 1.0)  # row 65 stays 1.0
            nc.vector.tensor_copy(out=decw16[64:65, :], in_=decw_sb[64:65, :])
            decw_re = decw16.rearrange("p (b t) -> p b t", b=BL)

            # ---------------- state ----------------
            # SY = [2h (0:64); w~0*dec_t (64); 1 (65)]
            sy = state.tile([66, BL], f16, tag="SY")
            nc.gpsimd.tensor_copy(out=sy[64:66, :], in_=decw_re[64:66, :, 0])
            st0_sb = state.tile([2 * P, BL], f16, tag="ST0")
            nc.sync.dma_start(out=st0_sb, in_=st0[:])
            nc.vector.tensor_copy(out=sy[0:P, :], in_=st0_sb[0:P, :])
            # cs lives inside tgx so one stt can compute [sV|sU] at once:
            # tgx = [i(0:4) f(4:8) o(8:12) g(12:16) cs(16:20)]
            tgx = state.tile([P, 5 * BL], f16, tag="TGX")
            cs = tgx[:, 4 * BL:5 * BL]
            nc.vector.tensor_copy(out=cs, in_=st0_sb[P:2 * P, :])

            # hw16[t, b] = H_b . w~[1:] + w~b   (f16, for num mms)
            hw16 = state.tile([T, BL], f16, tag="HW16")
            haug_all = state.tile([T, BL * M], f32r, tag="HAUGALL")
            nc.sync.dma_start(out=haug_all, in_=h_l[:].bitcast(f32r))
            HAUG = [haug_all[:, b * M:(b + 1) * M] for b in range(BL)]
            for b in range(BL):
                hsrc = hte_sb if b % 2 == 0 else hto_sb
                g = b // 2
                hw_ps = ps_aux.tile([T, 1], f32, tag="HWPS")
                nc.tensor.matmul(hw_ps, hsrc[:, g * T:(g + 1) * T], wt1h_sb,
                                 start=True, stop=True)
                nc.vector.tensor_scalar(
                    out=hw16[:, b:b + 1], in0=hw_ps, scalar1=wtb_sb,
                    scalar2=None, op0=OP.add)

            # UHe psum per group: [(j,m) 128, t 128], separate banks so the
            # two e-tanh ACTs don't serialize on the psum read-port hold
            uhe0 = ps_uhe0.tile([2 * M, T], f32, tag="UHE0")
            uhe1 = ps_uhe1.tile([2 * M, T], f32, tag="UHE1")
            uheg = [uhe0, uhe1]

            def uh_mms():
                # UH part of UHe (depends only on H) - starts the groups
                for g in range(NG):
                    nc.tensor.matmul(uheg[g], udTp_sb[:, 0:128],
                                     hte_sb[:, g * T:(g + 1) * T],
                                     start=True, stop=False)
                    nc.tensor.matmul(uheg[g], udTp_sb[:, 128:256],
                                     hto_sb[:, g * T:(g + 1) * T],
                                     start=False, stop=False)

            def c_mms():
                # W_dc . 2c part (after cs update)
                for g in range(NG):
                    for j in range(2):
                        b = 2 * g + j
                        nc.tensor.matmul(
                            uheg[g],
                            wdc2_sb[:, j * 128:(j + 1) * 128],
                            cs[:, b:b + 1].to_broadcast([P, T]),
                            start=False, stop=False)

            def h_mms(g):
                # W_dh . 2h part for group g - finishes UHe group g
                for j in range(2):
                    b = 2 * g + j
                    nc.tensor.matmul(
                        uheg[g],
                        wdh2_sb[:, j * 128:(j + 1) * 128],
                        sy[0:P, b:b + 1].to_broadcast([P, T]),
                        start=False, stop=(j == 1))

            # preamble: UHe(0) = UH + c-part (h-part comes in step 0)
            uh_mms()
            c_mms()

            # ---------------- loop tiles ----------------
            ES = state.tile([2 * M, NG * T], f16, tag="ES")
            s_sb = state.tile([T, BL], f16, tag="S")
            r_sb = state.tile([1, BL], f16, tag="R")
            rc_sb = state.tile([1, BL], f32, tag="RC")
            sVU = state.tile([P, 2 * BL], f16, tag="SVU")
            th = state.tile([P, BL], f16, tag="TH")

            def attention_front():
                """h-mms -> e-tanh -> lg -> exp; returns lg psum (read by exp
                already) and leaves s in s_sb."""
                # g0 first so e0 starts as early as possible; e1 reads the
                # other psum bank so there's no read-port serialization
                for g in (0, 1):
                    h_mms(g)
                for g in range(NG):
                    nc.scalar.activation(
                        out=ES[:, g * T:(g + 1) * T],
                        in_=uheg[g], func=AF.Tanh)
                lg = ps_lg.tile([T, BL], f32, tag="LG")
                for g in range(NG):
                    nc.tensor.matmul(lg[:, 2 * g:2 * g + 2],
                                     ES[:, g * T:(g + 1) * T], v2_sb,
                                     start=True, stop=True)
                nc.scalar.activation(out=s_sb, in_=lg, func=AF.Exp)

            def dennum():
                dn = ps_dn.tile([1, 2 * BL], f32, tag="DN")
                nc.tensor.matmul(dn[0:1, 0:BL], ones_col, s_sb,
                                 start=True, stop=True)
                for b in range(BL):
                    nc.tensor.matmul(dn[0:1, BL + b:BL + b + 1],
                                     hw16[:, b:b + 1], s_sb[:, b:b + 1],
                                     start=True, stop=True)
                return dn

            for t in range(T - 1):
                attention_front()
                # gates base: mm_A (h + dec + bias) - off the div chain
                # one accumulation group for the whole gps bank: first mm
                # start=True zeroes the region, last (mm_B k=3) stops it
                gps = ps_g.tile([P, 4 * BL], f32, tag="G")
                for k in range(4):
                    nc.tensor.matmul(gps[:, k * BL:(k + 1) * BL],
                                     whha_sb[:, k * P:(k + 1) * P], sy,
                                     start=(k == 0), stop=False)
                dn = dennum()
                # r = num / den  (DVE: recip then mult; divide is rejected
                # by the BIR verifier)
                nc.vector.reciprocal(out=rc_sb, in_=dn[0:1, 0:BL])
                nc.vector.tensor_tensor(
                    out=r_sb, in0=dn[0:1, BL:2 * BL], in1=rc_sb, op=OP.mult)
                # gates y~ part: rank-1 accumulate
                for k in range(4):
                    nc.tensor.matmul(gps[:, k * BL:(k + 1) * BL],
                                     wihb_sb[:, k * P:(k + 1) * P], r_sb,
                                     start=False, stop=(k == 3))
                # next step's UH part (waits e-reads via WAR)
                uh_mms()
                # LSTM pointwise tail, tgx gate order i,f,o,g + cs slot
                nc.scalar.activation(out=tgx[:, 0:4 * BL], in_=gps,
                                     func=AF.Tanh)
                t_o = tgx[:, 2 * BL:3 * BL]
                # one stt: [sV|sU] = ([i|f] + 1) * [g|cs]
                nc.vector.scalar_tensor_tensor(
                    out=sVU, in0=tgx[:, 0:2 * BL], scalar=1.0,
                    in1=tgx[:, 3 * BL:5 * BL], op0=OP.add, op1=OP.mult)
                nc.vector.scalar_tensor_tensor(
                    out=cs, in0=sVU[:, BL:2 * BL], in1=sVU[:, 0:BL],
                    scalar=0.5, op0=OP.mult, op1=OP.add)
                # c-part of next UHe now that cs is updated
                c_mms()
                nc.scalar.activation(out=th, in_=cs, func=AF.Tanh, scale=0.5)
                nc.vector.scalar_tensor_tensor(
                    out=sy[0:P, :], in0=t_o, scalar=1.0, in1=th,
                    op0=OP.add, op1=OP.mult)
                # decw for next step (WAR: waits mm_A read)
                nc.gpsimd.tensor_copy(out=sy[64:66, :],
                                      in_=decw_re[64:66, :, t + 1])

            # ---------------- final attend + outputs ----------------
            attention_front()
            dn = dennum()
            s_fr = loop_sb.tile([T, BL], f32, tag="SFR")
            nc.vector.tensor_copy(out=s_fr, in_=s_sb)
            # plain-f32 matmuls -> contiguous [M, BL] psum, DMA'd directly
            ctx_ps = ps_g.tile([M, BL], f32, tag="CTXF")
            for b in range(BL):
                nc.tensor.matmul(
                    ctx_ps[:, b:b + 1],
                    HAUG[b].bitcast(f32),
                    s_fr[:, b:b + 1],
                    start=(b == 0), stop=(b == 3))
            ctx_out = loop_sb.tile([M, BL], f32, tag="CTXOUT")
            nc.vector.tensor_copy(out=ctx_out, in_=ctx_ps)
            den_out = loop_sb.tile([1, BL], f32, tag="DENOUT")
            nc.vector.tensor_copy(out=den_out, in_=dn[0:1, 0:BL])
            oh_sb = loop_sb.tile([P, BL], f32, tag="OHSB")
            nc.vector.tensor_copy(out=oh_sb, in_=sy[0:P, :])
            nc.sync.dma_start(out=octx[:], in_=ctx_out)
            nc.gpsimd.dma_start(out=oden[:], in_=den_out)
            nc.scalar.dma_start(out=oh[:], in_=oh_sb)

    nc.finalize()
    return nc


def _pack_weights(W_d, U_d, v_d, w_tilde_W, w_tilde_b, W_ih, W_hh, b_ih, b_hh):
    f = np.float32
    # per-j lhsT is [64, 128] with the 64-col block at offset 64*j
    wdh2 = np.zeros((M, 4 * M), dtype=np.float16)
    wdc2 = np.zeros((M, 4 * M), dtype=np.float16)
    udTp = np.zeros((M, 4 * M), dtype=np.float16)
    for j in range(2):
        blk = slice(j * 128 + j * 64, j * 128 + (j + 1) * 64)
        wdh2[:, blk] = 0.5 * W_d[:, 0:M].T
        wdc2[:, blk] = 0.5 * W_d[:, M:2 * M].T
        udTp[:, blk] = U_d.T
    v2 = np.zeros((2 * M, NG), dtype=np.float16)
    v2[0:M, 0] = v_d[0]
    v2[M:2 * M, 1] = v_d[0]
    wt1h = np.ascontiguousarray(
        w_tilde_W[0, 1:M + 1].reshape(M, 1)).astype(np.float16)
    wtbbc = np.full((T, 1), w_tilde_b[0], dtype=f)
    bsum = (b_ih + b_hh).astype(f)
    wih = W_ih[:, 0].astype(f)
    w0 = np.float32(w_tilde_W[0, 0])
    # torch gate order i,f,g,o; our column order i,f,o,g.
    # sigmoid gates (i,f,o): pre-scale 0.5 (sigmoid(z) = 0.5 tanh(0.5 z)+0.5)
    # h input is 2h -> extra 0.5 on W_hh blocks.
    src = [0, 1, 3, 2]
    sig = [0.5, 0.5, 0.5, 1.0]
    whha = np.zeros((P + 2, 4 * P), dtype=np.float16)
    wihb = np.zeros((1, 4 * P), dtype=np.float16)
    for k in range(4):
        blk = slice(src[k] * P, (src[k] + 1) * P)
        whha[0:P, k * P:(k + 1) * P] = sig[k] * 0.5 * W_hh[blk].T
        whha[P, k * P:(k + 1) * P] = sig[k] * wih[blk]
        whha[P + 1, k * P:(k + 1) * P] = sig[k] * bsum[blk]
        wihb[0, k * P:(k + 1) * P] = sig[k] * wih[blk]
    wdsuite = np.concatenate([udTp, wdh2, wdc2], axis=1)
    return dict(wdsuite=wdsuite, v2=v2, wt1h=wt1h,
                wtbbc=wtbbc, whha=whha, wihb=wihb), w0


def kernel(H, dec_data, d_1, s_1, W_d, U_d, v_d, w_tilde_W, w_tilde_b,
           W_ih, W_hh, b_ih, b_hh, T=None):
    from concourse.bass_utils import run_bass_kernel_spmd

    H = np.asarray(H, dtype=np.float32)
    dec_data = np.asarray(dec_data, dtype=np.float32)
    d_1 = np.asarray(d_1, dtype=np.float32)
    s_1 = np.asarray(s_1, dtype=np.float32)

    if "nc" not in _STATE:
        _STATE["nc"] = _build_nc()
    nc = _STATE["nc"]

    wpack, w0 = _pack_weights(
        np.asarray(W_d, np.float32), np.asarray(U_d, np.float32),
        np.asarray(v_d, np.float32), np.asarray(w_tilde_W, np.float32),
        np.asarray(w_tilde_b, np.float32), np.asarray(W_ih, np.float32),
        np.asarray(W_hh, np.float32), np.asarray(b_ih, np.float32),
        np.asarray(b_hh, np.float32),
    )

    in_maps = []
    for core in range(NCORES):
        sl = slice(core * BL, (core + 1) * BL)
        hb = H[sl]                                              # [4, T, M]
        h_l = np.ascontiguousarray(
            hb.transpose(1, 0, 2).reshape(T, BL * M))           # [T, 4*M]
        ht = hb.transpose(0, 2, 1)                              # [4, M, T]
        hts = np.ascontiguousarray(np.concatenate(
            [ht[0], ht[2], ht[1], ht[3]], axis=1)).astype(np.float16)
        dec_l = np.ascontiguousarray(
            (w0 * dec_data[sl, :, 0]).reshape(1, BL * 128).astype(np.float32))
        st = np.concatenate(
            [2.0 * d_1[0, sl].T, 2.0 * s_1[0, sl].T], axis=0
        ).astype(np.float16)
        m = dict(wpack)
        m.update(h_l=h_l, hts=hts, dec_l=dec_l,
                 st0=np.ascontiguousarray(st))
        in_maps.append(m)

    res = run_bass_kernel_spmd(nc, in_maps, list(range(NCORES)))
    _STATE["last_results"] = res

    out = np.zeros((B, 1, P + M), dtype=np.float32)
    for core in range(NCORES):
        r = res.results[core]
        hv = r["oh"].T * 0.5                      # [4, 64]  (state was 2h)
        ctx = (r["octx"] / r["oden"]).T           # [4, 64]
        out[core * BL:(core + 1) * BL, 0, 0:P] = hv
        out[core * BL:(core + 1) * BL, 0, P:P + M] = ctx
    return out


# revision 4
# speedup vs baseline: 1.0546x; 1.0546x over previous
"""Trainium2 Bass kernel for nn_Decoder (attention-LSTM decoder recurrence).

v2: latency-oriented redesign. Per core: 4 batches, T-1=127 serial steps.

Critical chain per step:
  h-mms (PE: W_dh.2h bias accumulated into UHe psum, broadcast rhs)
  -> e0/e1 tanh (ACT [128,128], no bias operand)
  -> lg mms (PE, ES stationary) -> exp (ACT) -> den mm (PE)
  -> r = num/den (DVE divide) -> mm_B gates y~-part (PE) -> tg tanh (ACT)
  -> sU/sV/cs (Pool) -> th tanh (ACT) -> syh (Pool) -> next h-mms

Off-chain: UH recompute + c-part mms (PE), gates h/dec/bias part (mm_A, PE),
num mms (PE), decw copy (Pool).

e-layout: [(j,m) partitions, (g,t) free]; the attention bias W_d.q is
accumulated by PE into the e-input psum (rhs = state column broadcast over
t), eliminating the PSUM->SBUF bias copies of the baseline.

State: sy [66, BL] = [2h(0:64); w~0*dec_t(64); 1(65)], cs [64, BL] = 2c.
sigmoid(z) = 0.5 tanh(0.5 z) + 0.5 folded into packed weights.
"""

import numpy as np

B, T, M, P = 32, 128, 64, 64
NCORES = 8
BL = B // NCORES          # batches per core = 4
NG = 2                    # attention groups (pairs of batches)

_STATE = {}


def _build_nc():
    import contextlib

    import concourse.bacc as bacc
    import concourse.tile as tile
    from concourse import mybir

    f32 = mybir.dt.float32
    f32r = mybir.dt.float32r
    f16 = mybir.dt.float16
    AF = mybir.ActivationFunctionType
    OP = mybir.AluOpType

    nc = bacc.Bacc()

    # ---- per-core sharded data ----
    h_l = nc.declare_dram_parameter("h_l", [T, BL * M], f32, isOutput=False)
    hts = nc.declare_dram_parameter("hts", [M, 4 * T], f16, isOutput=False)
    dec_l = nc.declare_dram_parameter("dec_l", [1, BL * T], f32, isOutput=False)
    st0 = nc.declare_dram_parameter("st0", [2 * P, BL], f16, isOutput=False)
    # ---- replicated packed weights ----
    wdsuite = nc.declare_dram_parameter("wdsuite", [M, 12 * M], f16, isOutput=False)
    v2 = nc.declare_dram_parameter("v2", [2 * M, NG], f16, isOutput=False)
    wt1h = nc.declare_dram_parameter("wt1h", [M, 1], f16, isOutput=False)
    wtbbc = nc.declare_dram_parameter("wtbbc", [T, 1], f32, isOutput=False)
    whha = nc.declare_dram_parameter("whha", [P + 2, 4 * P], f16, isOutput=False)
    wihb = nc.declare_dram_parameter("wihb", [1, 4 * P], f16, isOutput=False)
    # ---- outputs ----
    oh = nc.declare_dram_parameter("oh", [P, BL], f32, isOutput=True)
    octx = nc.declare_dram_parameter("octx", [M, BL], f32, isOutput=True)
    oden = nc.declare_dram_parameter("oden", [1, BL], f32, isOutput=True)

    with tile.TileContext(nc) as tc:
        with contextlib.ExitStack() as ctx:
            consts = ctx.enter_context(tc.tile_pool(name="consts", bufs=1))
            state = ctx.enter_context(tc.tile_pool(name="state", bufs=1))
            loop_sb = ctx.enter_context(tc.tile_pool(name="loop_sb", bufs=1))
            rot = ctx.enter_context(tc.tile_pool(name="rot", bufs=2))
            ps_uhe0 = ctx.enter_context(
                tc.tile_pool(name="ps_uhe0", bufs=1, space="PSUM"))
            ps_uhe1 = ctx.enter_context(
                tc.tile_pool(name="ps_uhe1", bufs=1, space="PSUM"))
            ps_lg = ctx.enter_context(
                tc.tile_pool(name="ps_lg", bufs=1, space="PSUM"))
            ps_dn = ctx.enter_context(
                tc.tile_pool(name="ps_dn", bufs=1, space="PSUM"))
            ps_g = ctx.enter_context(
                tc.tile_pool(name="ps_g", bufs=1, space="PSUM"))
            ps_aux = ctx.enter_context(
                tc.tile_pool(name="ps_aux", bufs=1, space="PSUM"))

            # ---------------- constants ----------------
            # DMAs spread across SP/ACT/DVE queues; critical-path first.
            hts_sb = consts.tile([M, 4 * T], f16)
            nc.sync.dma_start(out=hts_sb, in_=hts[:])
            hte_sb = hts_sb[:, 0:2 * T]
            hto_sb = hts_sb[:, 2 * T:4 * T]
            wdsuite_sb = consts.tile([M, 12 * M], f16)
            nc.scalar.dma_start(out=wdsuite_sb, in_=wdsuite[:])
            udTp_sb = wdsuite_sb[:, 0:4 * M]
            wdh2_sb = wdsuite_sb[:, 4 * M:8 * M]
            wdc2_sb = wdsuite_sb[:, 8 * M:12 * M]
            st0_sb_dma = None
            v2_sb = consts.tile([2 * M, NG], f16)
            nc.gpsimd.dma_start(out=v2_sb, in_=v2[:])
            wt1h_sb = consts.tile([M, 1], f16)
            nc.gpsimd.dma_start(out=wt1h_sb, in_=wt1h[:])
            wtb_sb = consts.tile([T, 1], f32)
            nc.gpsimd.dma_start(out=wtb_sb, in_=wtbbc[:])
            whha_sb = consts.tile([P + 2, 4 * P], f16)
            nc.scalar.dma_start(out=whha_sb, in_=whha[:])
            wihb_sb = consts.tile([1, 4 * P], f16)
            nc.scalar.dma_start(out=wihb_sb, in_=wihb[:])
            ones_f = consts.tile([T, 1], f32)
            nc.vector.memset(ones_f, 1.0)
            ones_col = consts.tile([T, 1], f16)
            nc.vector.tensor_copy(out=ones_col, in_=ones_f)
            # dec (pre-multiplied by w~0 on host) lives at partition row 64
            decw_sb = consts.tile([66, BL * T], f32)
            nc.gpsimd.dma_start(out=decw_sb[64:65, :], in_=dec_l[:])
            decw16 = consts.tile([66, BL * T], f16)
            nc.vector.memset(decw16[64:66, :], 1.0)  # row 65 stays 1.0
            nc.vector.tensor_copy(out=decw16[64:65, :], in_=decw_sb[64:65, :])
            decw_re = decw16.rearrange("p (b t) -> p b t", b=BL)

            # ---------------- state ----------------
            # SY = [2h (0:64); w~0*dec_t (64); 1 (65)]
            sy = state.tile([66, BL], f16, tag="SY")
            nc.gpsimd.tensor_copy(out=sy[64:66, :], in_=decw_re[64:66, :, 0])
            st0_sb = state.tile([2 * P, BL], f16, tag="ST0")
            nc.sync.dma_start(out=st0_sb, in_=st0[:])
            nc.vector.tensor_copy(out=sy[0:P, :], in_=st0_sb[0:P, :])
            # cs lives inside tgx so one stt can compute [sV|sU] at once:
            # tgx = [i(0:4) f(4:8) o(8:12) g(12:16) cs(16:20)]
            tgx = state.tile([P, 5 * BL], f16, tag="TGX")
            cs = tgx[:, 4 * BL:5 * BL]
            nc.vector.tensor_copy(out=cs, in_=st0_sb[P:2 * P, :])

            # hw16[t, b] = H_b . w~[1:] + w~b   (f16, for num mms)
            hw16 = state.tile([T, BL], f16, tag="HW16")
            haug_all = state.tile([T, BL * M], f32r, tag="HAUGALL")
            nc.sync.dma_start(out=haug_all, in_=h_l[:].bitcast(f32r))
            HAUG = [haug_all[:, b * M:(b + 1) * M] for b in range(BL)]
            for b in range(BL):
                hsrc = hte_sb if b % 2 == 0 else hto_sb
                g = b // 2
                hw_ps = ps_aux.tile([T, 1], f32, tag="HWPS")
                nc.tensor.matmul(hw_ps, hsrc[:, g * T:(g + 1) * T], wt1h_sb,
                                 start=True, stop=True)
                nc.vector.tensor_scalar(
                    out=hw16[:, b:b + 1], in0=hw_ps, scalar1=wtb_sb,
                    scalar2=None, op0=OP.add)

            # UHe psum per group: [(j,m) 128, t 128], separate banks so the
            # two e-tanh ACTs don't serialize on the psum read-port hold
            uhe0 = ps_uhe0.tile([2 * M, T], f32, tag="UHE0")
            uhe1 = ps_uhe1.tile([2 * M, T], f32, tag="UHE1")
            uheg = [uhe0, uhe1]

            def uh_mms():
                # UH part of UHe (depends only on H) - starts the groups
                for g in range(NG):
                    nc.tensor.matmul(uheg[g], udTp_sb[:, 0:128],
                                     hte_sb[:, g * T:(g + 1) * T],
                                     start=True, stop=False)
                    nc.tensor.matmul(uheg[g], udTp_sb[:, 128:256],
                                     hto_sb[:, g * T:(g + 1) * T],
                                     start=False, stop=False)

            def c_mms():
                # W_dc . 2c part (after cs update)
                for g in range(NG):
                    for j in range(2):
                        b = 2 * g + j
                        nc.tensor.matmul(
                            uheg[g],
                            wdc2_sb[:, j * 128:(j + 1) * 128],
                            cs[:, b:b + 1].to_broadcast([P, T]),
                            start=False, stop=False)

            def h_mms(g):
                # W_dh . 2h part for group g - finishes UHe group g
                for j in range(2):
                    b = 2 * g + j
                    nc.tensor.matmul(
                        uheg[g],
                        wdh2_sb[:, j * 128:(j + 1) * 128],
                        sy[0:P, b:b + 1].to_broadcast([P, T]),
                        start=False, stop=(j == 1))

            # preamble: UHe(0) = UH + c-part (h-part comes in step 0)
            uh_mms()
            c_mms()

            # ---------------- loop tiles ----------------

            rc_sb = state.tile([1, BL], f32, tag="RC")


            def attention_front():
                """h-mms -> e-tanh -> lg -> exp; returns (ES, s_sb)."""
                ES = rot.tile([2 * M, NG * T], f16, tag="ES")
                s_sb = rot.tile([T, BL], f16, tag="S")
                # g0 first so e0 starts as early as possible; e1 reads the
                # other psum bank so there's no read-port serialization
                for g in (0, 1):
                    h_mms(g)
                for g in range(NG):
                    nc.scalar.activation(
                        out=ES[:, g * T:(g + 1) * T],
                        in_=uheg[g], func=AF.Tanh)
                lg = ps_lg.tile([T, BL], f32, tag="LG")
                for g in range(NG):
                    nc.tensor.matmul(lg[:, 2 * g:2 * g + 2],
                                     ES[:, g * T:(g + 1) * T], v2_sb,
                                     start=True, stop=True)
                nc.scalar.activation(out=s_sb, in_=lg, func=AF.Exp)
                return s_sb

            def dennum(s_sb):
                dn = ps_dn.tile([1, 2 * BL], f32, tag="DN")
                nc.tensor.matmul(dn[0:1, 0:BL], ones_col, s_sb,
                                 start=True, stop=True)
                for b in range(BL):
                    nc.tensor.matmul(dn[0:1, BL + b:BL + b + 1],
                                     hw16[:, b:b + 1], s_sb[:, b:b + 1],
                                     start=True, stop=True)
                return dn

            for t in range(T - 1):
                s_sb = attention_front()
                # gates base: mm_A (h + dec + bias) - off the div chain
                # one accumulation group for the whole gps bank: first mm
                # start=True zeroes the region, last (mm_B k=3) stops it
                gps = ps_g.tile([P, 4 * BL], f32, tag="G")
                for k in range(4):
                    nc.tensor.matmul(gps[:, k * BL:(k + 1) * BL],
                                     whha_sb[:, k * P:(k + 1) * P], sy,
                                     start=(k == 0), stop=False)
                dn = dennum(s_sb)
                r_sb = rot.tile([1, BL], f16, tag="R")
                sVU = rot.tile([P, 2 * BL], f16, tag="SVU")
                th = rot.tile([P, BL], f16, tag="TH")
                # r = num / den  (DVE: recip then mult; divide is rejected
                # by the BIR verifier)
                nc.vector.reciprocal(out=rc_sb, in_=dn[0:1, 0:BL])
                nc.vector.tensor_tensor(
                    out=r_sb, in0=dn[0:1, BL:2 * BL], in1=rc_sb, op=OP.mult)
                # gates y~ part: rank-1 accumulate
                for k in range(4):
                    nc.tensor.matmul(gps[:, k * BL:(k + 1) * BL],
                                     wihb_sb[:, k * P:(k + 1) * P], r_sb,
                                     start=False, stop=(k == 3))
                # next step's UH part (waits e-reads via WAR)
                uh_mms()
                # LSTM pointwise tail, tgx gate order i,f,o,g + cs slot
                nc.scalar.activation(out=tgx[:, 0:4 * BL], in_=gps,
                                     func=AF.Tanh)
                t_o = tgx[:, 2 * BL:3 * BL]
                # one stt: [sV|sU] = ([i|f] + 1) * [g|cs]
                nc.vector.scalar_tensor_tensor(
                    out=sVU, in0=tgx[:, 0:2 * BL], scalar=1.0,
                    in1=tgx[:, 3 * BL:5 * BL], op0=OP.add, op1=OP.mult)
                nc.vector.scalar_tensor_tensor(
                    out=cs, in0=sVU[:, BL:2 * BL], in1=sVU[:, 0:BL],
                    scalar=0.5, op0=OP.mult, op1=OP.add)
                # c-part of next UHe now that cs is updated
                c_mms()
                nc.scalar.activation(out=th, in_=cs, func=AF.Tanh, scale=0.5)
                nc.vector.scalar_tensor_tensor(
                    out=sy[0:P, :], in0=t_o, scalar=1.0, in1=th,
                    op0=OP.add, op1=OP.mult)
                # decw for next step (WAR: waits mm_A read)
                nc.gpsimd.tensor_copy(out=sy[64:66, :],
                                      in_=decw_re[64:66, :, t + 1])

            # ---------------- final attend + outputs ----------------
            s_sb = attention_front()
            dn = dennum(s_sb)
            s_fr = loop_sb.tile([T, BL], f32, tag="SFR")
            nc.vector.tensor_copy(out=s_fr, in_=s_sb)
            # reuse the gps psum allocation (same tag/shape) for the final
            # context so the pool stays at one bank
            gfin = ps_g.tile([P, 4 * BL], f32, tag="G")
            ctx_ps = gfin[:, 0:BL]
            for b in range(BL):
                nc.tensor.matmul(
                    ctx_ps[:, b:b + 1],
                    HAUG[b].bitcast(f32),
                    s_fr[:, b:b + 1],
                    start=(b == 0), stop=(b == 3))
            ctx_out = loop_sb.tile([M, BL], f32, tag="CTXOUT")
            nc.vector.tensor_copy(out=ctx_out, in_=ctx_ps)
            den_out = loop_sb.tile([1, BL], f32, tag="DENOUT")
            nc.vector.tensor_copy(out=den_out, in_=dn[0:1, 0:BL])
            oh_sb = loop_sb.tile([P, BL], f32, tag="OHSB")
            nc.vector.tensor_copy(out=oh_sb, in_=sy[0:P, :])
            nc.sync.dma_start(out=octx[:], in_=ctx_out)
            nc.gpsimd.dma_start(out=oden[:], in_=den_out)
            nc.scalar.dma_start(out=oh[:], in_=oh_sb)

    nc.finalize()
    return nc


def _pack_weights(W_d, U_d, v_d, w_tilde_W, w_tilde_b, W_ih, W_hh, b_ih, b_hh):
    f = np.float32
    # per-j lhsT is [64, 128] with the 64-col block at offset 64*j
    wdh2 = np.zeros((M, 4 * M), dtype=np.float16)
    wdc2 = np.zeros((M, 4 * M), dtype=np.float16)
    udTp = np.zeros((M, 4 * M), dtype=np.float16)
    for j in range(2):
        blk = slice(j * 128 + j * 64, j * 128 + (j + 1) * 64)
        wdh2[:, blk] = 0.5 * W_d[:, 0:M].T
        wdc2[:, blk] = 0.5 * W_d[:, M:2 * M].T
        udTp[:, blk] = U_d.T
    v2 = np.zeros((2 * M, NG), dtype=np.float16)
    v2[0:M, 0] = v_d[0]
    v2[M:2 * M, 1] = v_d[0]
    wt1h = np.ascontiguousarray(
        w_tilde_W[0, 1:M + 1].reshape(M, 1)).astype(np.float16)
    wtbbc = np.full((T, 1), w_tilde_b[0], dtype=f)
    bsum = (b_ih + b_hh).astype(f)
    wih = W_ih[:, 0].astype(f)
    w0 = np.float32(w_tilde_W[0, 0])
    # torch gate order i,f,g,o; our column order i,f,o,g.
    # sigmoid gates (i,f,o): pre-scale 0.5 (sigmoid(z) = 0.5 tanh(0.5 z)+0.5)
    # h input is 2h -> extra 0.5 on W_hh blocks.
    src = [0, 1, 3, 2]
    sig = [0.5, 0.5, 0.5, 1.0]
    whha = np.zeros((P + 2, 4 * P), dtype=np.float16)
    wihb = np.zeros((1, 4 * P), dtype=np.float16)
    for k in range(4):
        blk = slice(src[k] * P, (src[k] + 1) * P)
        whha[0:P, k * P:(k + 1) * P] = sig[k] * 0.5 * W_hh[blk].T
        whha[P, k * P:(k + 1) * P] = sig[k] * wih[blk]
        whha[P + 1, k * P:(k + 1) * P] = sig[k] * bsum[blk]
        wihb[0, k * P:(k + 1) * P] = sig[k] * wih[blk]
    wdsuite = np.concatenate([udTp, wdh2, wdc2], axis=1)
    return dict(wdsuite=wdsuite, v2=v2, wt1h=wt1h,
                wtbbc=wtbbc, whha=whha, wihb=wihb), w0


def kernel(H, dec_data, d_1, s_1, W_d, U_d, v_d, w_tilde_W, w_tilde_b,
           W_ih, W_hh, b_ih, b_hh, T=None):
    from concourse.bass_utils import run_bass_kernel_spmd

    H = np.asarray(H, dtype=np.float32)
    dec_data = np.asarray(dec_data, dtype=np.float32)
    d_1 = np.asarray(d_1, dtype=np.float32)
    s_1 = np.asarray(s_1, dtype=np.float32)

    if "nc" not in _STATE:
        _STATE["nc"] = _build_nc()
    nc = _STATE["nc"]

    wpack, w0 = _pack_weights(
        np.asarray(W_d, np.float32), np.asarray(U_d, np.float32),
        np.asarray(v_d, np.float32), np.asarray(w_tilde_W, np.float32),
        np.asarray(w_tilde_b, np.float32), np.asarray(W_ih, np.float32),
        np.asarray(W_hh, np.float32), np.asarray(b_ih, np.float32),
        np.asarray(b_hh, np.float32),
    )

    in_maps = []
    for core in range(NCORES):
        sl = slice(core * BL, (core + 1) * BL)
        hb = H[sl]                                              # [4, T, M]
        h_l = np.ascontiguousarray(
            hb.transpose(1, 0, 2).reshape(T, BL * M))           # [T, 4*M]
        ht = hb.transpose(0, 2, 1)                              # [4, M, T]
        hts = np.ascontiguousarray(np.concatenate(
            [ht[0], ht[2], ht[1], ht[3]], axis=1)).astype(np.float16)
        dec_l = np.ascontiguousarray(
            (w0 * dec_data[sl, :, 0]).reshape(1, BL * 128).astype(np.float32))
        st = np.concatenate(
            [2.0 * d_1[0, sl].T, 2.0 * s_1[0, sl].T], axis=0
        ).astype(np.float16)
        m = dict(wpack)
        m.update(h_l=h_l, hts=hts, dec_l=dec_l,
                 st0=np.ascontiguousarray(st))
        in_maps.append(m)

    res = run_bass_kernel_spmd(nc, in_maps, list(range(NCORES)))
    _STATE["last_results"] = res

    out = np.zeros((B, 1, P + M), dtype=np.float32)
    for core in range(NCORES):
        r = res.results[core]
        hv = r["oh"].T * 0.5                      # [4, 64]  (state was 2h)
        ctx = (r["octx"] / r["oden"]).T           # [4, 64]
        out[core * BL:(core + 1) * BL, 0, 0:P] = hv
        out[core * BL:(core + 1) * BL, 0, P:P + M] = ctx
    return out
